# revision 9
# baseline (speedup 1.0000x reference)
"""BasinCoupledAttention Trainium2 kernel.

Full inputs -> full output. Sharding: 2-way data parallel over batch x
4-way tensor parallel over heads (4 heads / core, 8 cores total).

Per-core device program (b = core//4, g = core%4, heads 4g..4g+3):
  - x_b^T [1024, 2048] resident in SBUF (d on partitions), fp16
  - q^T, k^T = (Wq_g^T x^T, Wk_g^T x^T) in [d', s] layout, fp16; the
    per-head sigmoid basin gate and 1/sqrt(dh) are folded into Wq_g /
    bq_g on the host
  - v = x Wv_g in [s, d'] layout with a ones-column per head, fp32r
  - attention head-pair interleaved: S^T[j,i] = k_h^T(jb)^T q_h^T (ib)
    via fp16 matmul; exp on ACT (no max subtraction: |scores| <= ~6),
    fp32r out; causal structurally (skip blocks above the diagonal,
    fp16 triangular mask multiply on diagonal squares);
    [out^T; rowsum] accumulated via v1 = [v | 1] fp32r matmuls
  - normalize by broadcast(1/rowsum); out_partial = O^T' Wo per s-chunk
Host sums the 4 per-core partials for each batch and adds bo.
"""

import sys

if "/opt/trn_rl_repo" not in sys.path:
    sys.path.insert(0, "/opt/trn_rl_repo")

import numpy as np

D = 1024          # d_model
S = 2048          # sequence
B = 2             # batch
HL = 4            # heads per core
DL = 256          # d' columns per core (HL * 64)
DH = 64           # head dim
N_CORES = 8

_CACHE = {}

PHASES = ("dma", "v", "qk", "attn", "outp")


def _build_program(reps=1, stop_after="all", loop_only=None):
    import concourse.bacc as bacc
    import concourse.mybir as mybir
    import concourse.tile as tile

    f32 = mybir.dt.float32
    f16 = mybir.dt.float16
    f32r = mybir.dt.float32r
    Exp = mybir.ActivationFunctionType.Exp

    nc = bacc.Bacc("TRN2", target_bir_lowering=False, debug=False)

    xt_d = nc.dram_tensor("xt", [D, S], f16, kind="ExternalInput").ap()
    wq_d = nc.dram_tensor("wq", [D, DL], f16, kind="ExternalInput").ap()
    wk_d = nc.dram_tensor("wk", [D, DL], f16, kind="ExternalInput").ap()
    wv_d = nc.dram_tensor("wv", [D, DL], f16, kind="ExternalInput").ap()
    wo_d = nc.dram_tensor("wo", [DL, D], f16, kind="ExternalInput").ap()
    bq_d = nc.dram_tensor("bq", [DL, 1], f32, kind="ExternalInput").ap()
    bk_d = nc.dram_tensor("bk", [DL, 1], f32, kind="ExternalInput").ap()
    # bv1 holds [h, 65]: cols 0..63 = bv bias for that head, col 64 = 1.0
    bv_d = nc.dram_tensor("bv", [1, HL * 65], f32, kind="ExternalInput").ap()
    tri_d = nc.dram_tensor("tri", [128, 128], f16, kind="ExternalInput").ap()
    out_d = nc.dram_tensor("out", [S, D], f32, kind="ExternalOutput").ap()

    with tile.TileContext(nc) as tc:
        with (
            tc.tile_pool(name="persist", bufs=1) as pp,
            tc.tile_pool(name="work", bufs=4) as wp,
            tc.tile_pool(name="ps_a", bufs=4, space="PSUM") as ps_a,
            tc.tile_pool(name="ps_av", bufs=3, space="PSUM") as ps_av,
        ):
            st = {}

            def ph_dma():
                xt_sb = [pp.tile([128, S], f16, name=f"xt{k}", tag=f"xt{k}")
                         for k in range(8)]
                for k in range(8):
                    nc.sync.dma_start(out=xt_sb[k],
                                      in_=xt_d[k * 128:(k + 1) * 128, :])
                wq_sb = [pp.tile([128, DL], f16, name=f"wq{k}", tag=f"wq{k}")
                         for k in range(8)]
                wk_sb = [pp.tile([128, DL], f16, name=f"wk{k}", tag=f"wk{k}")
                         for k in range(8)]
                wv_sb = [pp.tile([128, DL], f16, name=f"wv{k}", tag=f"wv{k}")
                         for k in range(8)]
                for k in range(8):
                    sl = slice(k * 128, (k + 1) * 128)
                    nc.sync.dma_start(out=wq_sb[k], in_=wq_d[sl, :])
                    nc.sync.dma_start(out=wk_sb[k], in_=wk_d[sl, :])
                    nc.sync.dma_start(out=wv_sb[k], in_=wv_d[sl, :])
                wo_sb = [pp.tile([128, D], f16, name=f"wo{t}", tag=f"wo{t}")
                        for t in range(2)]
                bq_sb = [pp.tile([128, 1], f32, name=f"bq{t}", tag=f"bq{t}")
                         for t in range(2)]
                bk_sb = [pp.tile([128, 1], f32, name=f"bk{t}", tag=f"bk{t}")
                         for t in range(2)]
                for t in range(2):
                    sl = slice(t * 128, (t + 1) * 128)
                    nc.sync.dma_start(out=wo_sb[t], in_=wo_d[sl, :])
                    nc.sync.dma_start(out=bq_sb[t], in_=bq_d[sl, :])
                    nc.sync.dma_start(out=bk_sb[t], in_=bk_d[sl, :])
                bv_sb = pp.tile([128, HL * 65], f32, name="bv", tag="bv")
                nc.sync.dma_start(out=bv_sb,
                                  in_=bv_d.to_broadcast([128, HL * 65]))
                tri_sb = pp.tile([128, 128], f16, name="tri", tag="tri")
                nc.sync.dma_start(out=tri_sb, in_=tri_d)
                st.update(xt=xt_sb, wq=wq_sb, wk=wk_sb, wv=wv_sb, wo=wo_sb,
                          bq=bq_sb, bk=bk_sb, tri=tri_sb,
                          bv_r=bv_sb.rearrange("p (h c) -> p h c", h=HL))

            def ph_v():
                v1_sb = [pp.tile([128, HL, 65], f32r, name=f"v1_{s_}",
                                 tag=f"v1_{s_}") for s_ in range(16)]
                for sc in range(16):
                    ps_v = ps_a.tile([128, DL], f32, name="ps_v", tag="ps")
                    for kd in range(8):
                        nc.tensor.matmul(
                            ps_v,
                            lhsT=st["xt"][kd][:, sc * 128:(sc + 1) * 128],
                            rhs=st["wv"][kd],
                            start=(kd == 0), stop=(kd == 7),
                        )
                    nc.vector.tensor_tensor(
                        out=v1_sb[sc][:, :, 0:64],
                        in0=ps_v.rearrange("p (h d) -> p h d", h=HL),
                        in1=st["bv_r"][:, :, 0:64],
                        op=mybir.AluOpType.add,
                    )
                    nc.vector.tensor_copy(v1_sb[sc][:, :, 64:65],
                                          st["bv_r"][:, :, 64:65])
                st["v1"] = v1_sb

            def ph_qk():
                qT_sb = [pp.tile([128, S], f16, name=f"qT{t}", tag=f"qT{t}")
                         for t in range(2)]
                kT_sb = [pp.tile([128, S], f16, name=f"kT{t}", tag=f"kT{t}")
                         for t in range(2)]
                for t in range(2):
                    for w_sb, b_sb, dst in ((st["wq"], st["bq"], qT_sb),
                                            (st["wk"], st["bk"], kT_sb)):
                        for nb in range(4):
                            ps_p = ps_a.tile([128, 512], f32, name="ps_p",
                                             tag="ps")
                            for kd in range(8):
                                nc.tensor.matmul(
                                    ps_p,
                                    lhsT=w_sb[kd][:, t * 128:(t + 1) * 128],
                                    rhs=st["xt"][kd][:, nb * 512:(nb + 1) * 512],
                                    start=(kd == 0), stop=(kd == 7),
                                )
                            nc.vector.tensor_scalar_add(
                                out=dst[t][:, nb * 512:(nb + 1) * 512],
                                in0=ps_p, scalar1=b_sb[t],
                            )
                st["qT"], st["kT"] = qT_sb, kT_sb

            def ph_attn():
                oT_sb = st.get("oT")
                if oT_sb is None:
                    oT_sb = [pp.tile([128, S], f16, name=f"oT{t}",
                                     tag=f"oT{t}") for t in range(2)]
                    st["oT"] = oT_sb
                v1_sb, qT_sb, kT_sb = st["v1"], st["qT"], st["kT"]
                tri_sb = st["tri"]
                for t in range(2):           # head pair (2t, 2t+1)
                    for ib in range(4):
                        njb = 4 * ib + 4
                        ps_o = {
                            0: ps_av.tile([65, 512], f32, name="ps_o0",
                                          tag="av"),
                            1: ps_av.tile([65, 512], f32, name="ps_o1",
                                          tag="av"),
                        }
                        pend = []

                        def flush_av(limit):
                            while len(pend) > limit:
                                h, jb, off, a_t = pend.pop(0)
                                nc.tensor.matmul(
                                    ps_o[h][:, off:512],
                                    lhsT=v1_sb[jb][:, 2 * t + h, :],
                                    rhs=a_t[:, off:512],
                                    start=(jb == 0), stop=(jb == njb - 1),
                                    skip_group_check=True,
                                )

                        for jb in range(njb):
                            diag = jb >= 4 * ib
                            off = jb * 128 - ib * 512 if diag else 0
                            for h in (0, 1):
                                bp = h * 64
                                ps_s = ps_a.tile([128, 512], f32,
                                                 name="ps_s", tag="ps")
                                nc.tensor.matmul(
                                    ps_s,
                                    lhsT=kT_sb[t][bp:bp + 64,
                                                  jb * 128:(jb + 1) * 128],
                                    rhs=qT_sb[t][bp:bp + 64,
                                                 ib * 512:(ib + 1) * 512],
                                    start=True, stop=True,
                                )
                                a_t = wp.tile([128, 512], f32r, name="a_t",
                                              tag="a", bufs=6)
                                nc.scalar.activation(a_t[:, off:512],
                                                     ps_s[:, off:512], Exp)
                                if diag:
                                    nc.vector.tensor_mul(
                                        a_t[:, off:off + 128],
                                        a_t[:, off:off + 128], tri_sb)
                                pend.append((h, jb, off, a_t))
                                flush_av(3)
                        flush_av(0)
                        for h in (0, 1):
                            bp = h * 64
                            recip = wp.tile([1, 512], f32, name="recip",
                                            tag="rc", bufs=2)
                            nc.vector.reciprocal(recip, ps_o[h][64:65, :])
                            rb = wp.tile([64, 512], f32, name="rb", tag="rb",
                                         bufs=2)
                            nc.gpsimd.partition_broadcast(rb, recip)
                            nc.vector.tensor_mul(
                                oT_sb[t][bp:bp + 64,
                                         ib * 512:(ib + 1) * 512],
                                ps_o[h][0:64, :], rb)

            def ph_outp():
                oT_sb, wo_sb = st["oT"], st["wo"]
                for sc in range(16):
                    o_sb = wp.tile([128, D], f32, name="o_sb", tag="osb",
                                   bufs=3)
                    for n in range(2):
                        ps_f = ps_a.tile([128, 512], f32, name="ps_f",
                                         tag="ps")
                        for t in range(2):
                            nc.tensor.matmul(
                                ps_f,
                                lhsT=oT_sb[t][:, sc * 128:(sc + 1) * 128],
                                rhs=wo_sb[t][:, n * 512:(n + 1) * 512],
                                start=(t == 0), stop=(t == 1),
                            )
                        nc.vector.tensor_copy(o_sb[:, n * 512:(n + 1) * 512],
                                              ps_f)
                    nc.sync.dma_start(out=out_d[sc * 128:(sc + 1) * 128, :],
                                      in_=o_sb)

            phase_fns = {"dma": ph_dma, "v": ph_v, "qk": ph_qk,
                         "attn": ph_attn, "outp": ph_outp}
            if stop_after == "all":
                active = list(PHASES)
            else:
                active = list(PHASES[:PHASES.index(stop_after) + 1])

            def make_loop():
                return tc.For_i(0, reps, 1,
                                hint_engines=(mybir.EngineType.PE,
                                              mybir.EngineType.Activation,
                                              mybir.EngineType.DVE))

            if loop_only is None:
                if reps == 1:
                    for name in active:
                        phase_fns[name]()
                else:
                    with make_loop():
                        for name in active:
                            phase_fns[name]()
            else:
                assert loop_only in active
                for name in active[:active.index(loop_only)]:
                    phase_fns[name]()
                with make_loop():
                    phase_fns[loop_only]()
                for name in active[active.index(loop_only) + 1:]:
                    phase_fns[name]()

    nc.compile()
    return nc


def _bv1(bv_slice):
    """[256] head-local v bias -> [1, 4*65] with a 1.0 ones-column."""
    out = np.zeros((1, HL * 65), np.float32)
    for h in range(HL):
        out[0, h * 65:h * 65 + 64] = bv_slice[h * 64:(h + 1) * 64]
        out[0, h * 65 + 64] = 1.0
    return out


def _prepare_in_maps(inputs):
    x = np.asarray(inputs["x"], np.float32)
    basin = np.asarray(inputs["basin"], np.float32)
    Wq = np.asarray(inputs["Wq"], np.float32)
    bq = np.asarray(inputs["bq"], np.float32)
    Wk = np.asarray(inputs["Wk"], np.float32)
    bk = np.asarray(inputs["bk"], np.float32)
    Wv = np.asarray(inputs["Wv"], np.float32)
    bv = np.asarray(inputs["bv"], np.float32)
    Wo = np.asarray(inputs["Wo"], np.float32)
    Wb = np.asarray(inputs["Wb"], np.float32)
    bb = np.asarray(inputs["bb"], np.float32)

    gate = 1.0 / (1.0 + np.exp(-(basin @ Wb + bb)))          # [16]
    scale = (gate / np.sqrt(float(DH))).astype(np.float32)   # [16]
    colscale = np.repeat(scale, DH)                          # [1024]
    Wq_s = (Wq * colscale[None, :]).astype(np.float32)
    bq_s = (bq * colscale).astype(np.float32)

    xt_all = [np.ascontiguousarray(x[b].T) for b in range(B)]
    tri = np.ascontiguousarray(np.triu(np.ones((128, 128), np.float16)))

    in_maps = []
    for c in range(N_CORES):
        b, g = divmod(c, 4)
        sl = slice(g * DL, (g + 1) * DL)
        in_maps.append({
            "xt": xt_all[b].astype(np.float16),
            "wq": np.ascontiguousarray(Wq_s[:, sl]).astype(np.float16),
            "wk": np.ascontiguousarray(Wk[:, sl]).astype(np.float16),
            "wv": np.ascontiguousarray(Wv[:, sl]).astype(np.float16),
            "wo": np.ascontiguousarray(Wo[sl, :]).astype(np.float16),
            "bq": np.ascontiguousarray(bq_s[sl]).reshape(DL, 1),
            "bk": np.ascontiguousarray(bk[sl]).reshape(DL, 1),
            "bv": _bv1(bv[sl]),
            "tri": tri,
        })
    return in_maps


def _run(inputs, trace=False):
    from concourse.bass_utils import run_bass_kernel_spmd

    nc = _CACHE.get("nc")
    if nc is None:
        nc = _build_program()
        _CACHE["nc"] = nc
    in_maps = _prepare_in_maps(inputs)
    res = run_bass_kernel_spmd(nc, in_maps, core_ids=list(range(N_CORES)),
                               trace=trace)
    bo = np.asarray(inputs["bo"], np.float32)
    out = np.zeros((B, S, D), np.float32)
    for c in range(N_CORES):
        out[c // 4] += res.results[c]["out"]
    out += bo[None, None, :]
    return out, res


def kernel(**inputs):
    out, _ = _run(inputs, trace=False)
    return out


# revision 10
# speedup vs baseline: 1.4923x; 1.4923x over previous
"""BasinCoupledAttention Trainium2 kernel.

Full inputs -> full output. Sharding: 2-way data parallel over batch x
4-way tensor parallel over heads (4 heads / core, 8 cores total).

Per-core device program (b = core//4, g = core%4, heads 4g..4g+3):
  - x_b^T [1024, 2048] resident in SBUF (d on partitions), fp16
  - q^T, k^T = (Wq_g^T x^T, Wk_g^T x^T) in [d', s] layout, fp16; the
    per-head sigmoid basin gate and 1/sqrt(dh) are folded into Wq_g /
    bq_g on the host
  - v = x Wv_g in [s, d'] layout with a ones-column per head, fp32r
  - attention head-pair interleaved: S^T[j,i] = k_h^T(jb)^T q_h^T (ib)
    via fp16 matmul; exp on ACT (no max subtraction: |scores| <= ~6),
    fp32r out; causal structurally (skip blocks above the diagonal,
    fp16 triangular mask multiply on diagonal squares);
    [out^T; rowsum] accumulated via v1 = [v | 1] fp32r matmuls
  - normalize by broadcast(1/rowsum); out_partial = O^T' Wo per s-chunk
Host sums the 4 per-core partials for each batch and adds bo.
"""

import sys

if "/opt/trn_rl_repo" not in sys.path:
    sys.path.insert(0, "/opt/trn_rl_repo")

import numpy as np

D = 1024          # d_model
S = 2048          # sequence
B = 2             # batch
HL = 4            # heads per core
DL = 256          # d' columns per core (HL * 64)
DH = 64           # head dim
N_CORES = 8

_CACHE = {}

PHASES = ("dma", "v", "qk", "attn", "outp")


def _build_program(reps=1, stop_after="all", loop_only=None):
    import concourse.bacc as bacc
    import concourse.mybir as mybir
    import concourse.tile as tile

    f32 = mybir.dt.float32
    f16 = mybir.dt.float16
    f32r = mybir.dt.float32r
    Exp = mybir.ActivationFunctionType.Exp

    nc = bacc.Bacc("TRN2", target_bir_lowering=False, debug=False)

    xt_d = nc.dram_tensor("xt", [D, S], f16, kind="ExternalInput").ap()
    wq_d = nc.dram_tensor("wq", [D, DL], f16, kind="ExternalInput").ap()
    wk_d = nc.dram_tensor("wk", [D, DL], f16, kind="ExternalInput").ap()
    wv_d = nc.dram_tensor("wv", [D, DL], f16, kind="ExternalInput").ap()
    wo_d = nc.dram_tensor("wo", [DL, D], f16, kind="ExternalInput").ap()
    bq_d = nc.dram_tensor("bq", [DL, 1], f32, kind="ExternalInput").ap()
    bk_d = nc.dram_tensor("bk", [DL, 1], f32, kind="ExternalInput").ap()
    # bv1 holds [h, 65]: cols 0..63 = bv bias for that head, col 64 = 1.0
    bv_d = nc.dram_tensor("bv", [1, HL * 65], f32, kind="ExternalInput").ap()
    tri_d = nc.dram_tensor("tri", [128, 128], f16, kind="ExternalInput").ap()
    out_d = nc.dram_tensor("out", [S, D], f32, kind="ExternalOutput").ap()

    with tile.TileContext(nc) as tc:
        with (
            tc.tile_pool(name="persist", bufs=1) as pp,
            tc.tile_pool(name="work", bufs=4) as wp,
            tc.tile_pool(name="ps_a", bufs=3, space="PSUM") as ps_a,
            tc.tile_pool(name="ps_av", bufs=2, space="PSUM") as ps_av,
        ):
            st = {}

            def ph_dma():
                xt_sb = [pp.tile([128, S], f16, name=f"xt{k}", tag=f"xt{k}")
                         for k in range(8)]
                for k in range(8):
                    nc.sync.dma_start(out=xt_sb[k],
                                      in_=xt_d[k * 128:(k + 1) * 128, :])
                wq_sb = [pp.tile([128, DL], f16, name=f"wq{k}", tag=f"wq{k}")
                         for k in range(8)]
                wk_sb = [pp.tile([128, DL], f16, name=f"wk{k}", tag=f"wk{k}")
                         for k in range(8)]
                wv_sb = [pp.tile([128, DL], f16, name=f"wv{k}", tag=f"wv{k}")
                         for k in range(8)]
                for k in range(8):
                    sl = slice(k * 128, (k + 1) * 128)
                    nc.sync.dma_start(out=wq_sb[k], in_=wq_d[sl, :])
                    nc.sync.dma_start(out=wk_sb[k], in_=wk_d[sl, :])
                    nc.sync.dma_start(out=wv_sb[k], in_=wv_d[sl, :])
                wo_sb = [pp.tile([128, D], f16, name=f"wo{t}", tag=f"wo{t}")
                        for t in range(2)]
                bq_sb = [pp.tile([128, 1], f32, name=f"bq{t}", tag=f"bq{t}")
                         for t in range(2)]
                bk_sb = [pp.tile([128, 1], f32, name=f"bk{t}", tag=f"bk{t}")
                         for t in range(2)]
                for t in range(2):
                    sl = slice(t * 128, (t + 1) * 128)
                    nc.sync.dma_start(out=wo_sb[t], in_=wo_d[sl, :])
                    nc.sync.dma_start(out=bq_sb[t], in_=bq_d[sl, :])
                    nc.sync.dma_start(out=bk_sb[t], in_=bk_d[sl, :])
                bv_sb = pp.tile([128, HL * 65], f32, name="bv", tag="bv")
                nc.sync.dma_start(out=bv_sb,
                                  in_=bv_d.to_broadcast([128, HL * 65]))
                tri_sb = pp.tile([128, 128], f16, name="tri", tag="tri")
                nc.sync.dma_start(out=tri_sb, in_=tri_d)
                st.update(xt=xt_sb, wq=wq_sb, wk=wk_sb, wv=wv_sb, wo=wo_sb,
                          bq=bq_sb, bk=bk_sb, tri=tri_sb,
                          bv_r=bv_sb.rearrange("p (h c) -> p h c", h=HL))

            def ph_v():
                v1_sb = [pp.tile([128, HL, 65], f32r, name=f"v1_{s_}",
                                 tag=f"v1_{s_}") for s_ in range(16)]
                for sc in range(16):
                    ps_v = ps_a.tile([128, DL], f32, name="ps_v", tag="ps")
                    for kd in range(8):
                        nc.tensor.matmul(
                            ps_v,
                            lhsT=st["xt"][kd][:, sc * 128:(sc + 1) * 128],
                            rhs=st["wv"][kd],
                            start=(kd == 0), stop=(kd == 7),
                        )
                    nc.vector.tensor_tensor(
                        out=v1_sb[sc][:, :, 0:64],
                        in0=ps_v.rearrange("p (h d) -> p h d", h=HL),
                        in1=st["bv_r"][:, :, 0:64],
                        op=mybir.AluOpType.add,
                    )
                    nc.vector.tensor_copy(v1_sb[sc][:, :, 64:65],
                                          st["bv_r"][:, :, 64:65])
                st["v1"] = v1_sb

            def ph_qk():
                qT_sb = [pp.tile([128, S], f16, name=f"qT{t}", tag=f"qT{t}")
                         for t in range(2)]
                kT_sb = [pp.tile([128, S], f16, name=f"kT{t}", tag=f"kT{t}")
                         for t in range(2)]
                for t in range(2):
                    for w_sb, b_sb, dst in ((st["wq"], st["bq"], qT_sb),
                                            (st["wk"], st["bk"], kT_sb)):
                        for nb in range(4):
                            ps_p = ps_a.tile([128, 512], f32, name="ps_p",
                                             tag="ps")
                            for kd in range(8):
                                nc.tensor.matmul(
                                    ps_p,
                                    lhsT=w_sb[kd][:, t * 128:(t + 1) * 128],
                                    rhs=st["xt"][kd][:, nb * 512:(nb + 1) * 512],
                                    start=(kd == 0), stop=(kd == 7),
                                )
                            nc.vector.tensor_scalar_add(
                                out=dst[t][:, nb * 512:(nb + 1) * 512],
                                in0=ps_p, scalar1=b_sb[t],
                            )
                st["qT"], st["kT"] = qT_sb, kT_sb

            def ph_attn():
                oT_sb = st.get("oT")
                if oT_sb is None:
                    oT_sb = [pp.tile([128, S], f16, name=f"oT{t}",
                                     tag=f"oT{t}") for t in range(2)]
                    st["oT"] = oT_sb
                v1_sb, qT_sb, kT_sb = st["v1"], st["qT"], st["kT"]
                tri_sb = st["tri"]
                for t in range(2):           # head pair (2t, 2t+1)
                    for ib in range(4):
                        njb = 4 * ib + 4
                        ps_o = {
                            0: ps_av.tile([65, 512], f32, name="ps_o0",
                                          tag="av"),
                            1: ps_av.tile([65, 512], f32, name="ps_o1",
                                          tag="av"),
                        }
                        pend = []

                        def flush_av(limit):
                            while len(pend) > limit:
                                h, jb, off, a_t = pend.pop(0)
                                nc.tensor.matmul(
                                    ps_o[h][:, off:512],
                                    lhsT=v1_sb[jb][:, 2 * t + h, :],
                                    rhs=a_t[:, 512 * h + off:512 * (h + 1)],
                                    start=(jb == 0), stop=(jb == njb - 1),
                                    skip_group_check=True,
                                )

                        for jb in range(njb):
                            diag = jb >= 4 * ib
                            off = jb * 128 - ib * 512 if diag else 0
                            ps_s = ps_a.tile([128, 1024], f32,
                                             name="ps_s", tag="ps")
                            for h in (0, 1):
                                bp = h * 64
                                nc.tensor.matmul(
                                    ps_s[:, 512 * h:512 * (h + 1)],
                                    lhsT=kT_sb[t][bp:bp + 64,
                                                  jb * 128:(jb + 1) * 128],
                                    rhs=qT_sb[t][bp:bp + 64,
                                                 ib * 512:(ib + 1) * 512],
                                    start=True, stop=True,
                                )
                            a_t = wp.tile([128, 1024], f32r, name="a_t",
                                          tag="a", bufs=4)
                            nc.scalar.activation(a_t[:, off:1024],
                                                 ps_s[:, off:1024], Exp)
                            if diag:
                                for h in (0, 1):
                                    nc.vector.tensor_mul(
                                        a_t[:, 512 * h + off:512 * h + off + 128],
                                        a_t[:, 512 * h + off:512 * h + off + 128],
                                        tri_sb)
                            pend.append((0, jb, off, a_t))
                            pend.append((1, jb, off, a_t))
                            flush_av(3)
                        flush_av(0)
                        for h in (0, 1):
                            bp = h * 64
                            recip = wp.tile([1, 512], f32, name="recip",
                                            tag="rc", bufs=2)
                            nc.vector.reciprocal(recip, ps_o[h][64:65, :])
                            rb = wp.tile([64, 512], f32, name="rb", tag="rb",
                                         bufs=2)
                            nc.gpsimd.partition_broadcast(rb, recip)
                            nc.vector.tensor_mul(
                                oT_sb[t][bp:bp + 64,
                                         ib * 512:(ib + 1) * 512],
                                ps_o[h][0:64, :], rb)

            def ph_outp():
                oT_sb, wo_sb = st["oT"], st["wo"]
                for sc in range(16):
                    o_sb = wp.tile([128, D], f32, name="o_sb", tag="osb",
                                   bufs=3)
                    for n in range(2):
                        ps_f = ps_a.tile([128, 512], f32, name="ps_f",
                                         tag="ps")
                        for t in range(2):
                            nc.tensor.matmul(
                                ps_f,
                                lhsT=oT_sb[t][:, sc * 128:(sc + 1) * 128],
                                rhs=wo_sb[t][:, n * 512:(n + 1) * 512],
                                start=(t == 0), stop=(t == 1),
                            )
                        nc.vector.tensor_copy(o_sb[:, n * 512:(n + 1) * 512],
                                              ps_f)
                    nc.sync.dma_start(out=out_d[sc * 128:(sc + 1) * 128, :],
                                      in_=o_sb)

            phase_fns = {"dma": ph_dma, "v": ph_v, "qk": ph_qk,
                         "attn": ph_attn, "outp": ph_outp}
            if stop_after == "all":
                active = list(PHASES)
            else:
                active = list(PHASES[:PHASES.index(stop_after) + 1])

            def make_loop():
                return tc.For_i(0, reps, 1,
                                hint_engines=(mybir.EngineType.PE,
                                              mybir.EngineType.Activation,
                                              mybir.EngineType.DVE))

            if loop_only is None:
                if reps == 1:
                    for name in active:
                        phase_fns[name]()
                else:
                    with make_loop():
                        for name in active:
                            phase_fns[name]()
            else:
                assert loop_only in active
                for name in active[:active.index(loop_only)]:
                    phase_fns[name]()
                with make_loop():
                    phase_fns[loop_only]()
                for name in active[active.index(loop_only) + 1:]:
                    phase_fns[name]()

    nc.compile()
    return nc


def _bv1(bv_slice):
    """[256] head-local v bias -> [1, 4*65] with a 1.0 ones-column."""
    out = np.zeros((1, HL * 65), np.float32)
    for h in range(HL):
        out[0, h * 65:h * 65 + 64] = bv_slice[h * 64:(h + 1) * 64]
        out[0, h * 65 + 64] = 1.0
    return out


def _prepare_in_maps(inputs):
    x = np.asarray(inputs["x"], np.float32)
    basin = np.asarray(inputs["basin"], np.float32)
    Wq = np.asarray(inputs["Wq"], np.float32)
    bq = np.asarray(inputs["bq"], np.float32)
    Wk = np.asarray(inputs["Wk"], np.float32)
    bk = np.asarray(inputs["bk"], np.float32)
    Wv = np.asarray(inputs["Wv"], np.float32)
    bv = np.asarray(inputs["bv"], np.float32)
    Wo = np.asarray(inputs["Wo"], np.float32)
    Wb = np.asarray(inputs["Wb"], np.float32)
    bb = np.asarray(inputs["bb"], np.float32)

    gate = 1.0 / (1.0 + np.exp(-(basin @ Wb + bb)))          # [16]
    scale = (gate / np.sqrt(float(DH))).astype(np.float32)   # [16]
    colscale = np.repeat(scale, DH)                          # [1024]
    Wq_s = (Wq * colscale[None, :]).astype(np.float32)
    bq_s = (bq * colscale).astype(np.float32)

    xt_all = [np.ascontiguousarray(x[b].T) for b in range(B)]
    tri = np.ascontiguousarray(np.triu(np.ones((128, 128), np.float16)))

    in_maps = []
    for c in range(N_CORES):
        b, g = divmod(c, 4)
        sl = slice(g * DL, (g + 1) * DL)
        in_maps.append({
            "xt": xt_all[b].astype(np.float16),
            "wq": np.ascontiguousarray(Wq_s[:, sl]).astype(np.float16),
            "wk": np.ascontiguousarray(Wk[:, sl]).astype(np.float16),
            "wv": np.ascontiguousarray(Wv[:, sl]).astype(np.float16),
            "wo": np.ascontiguousarray(Wo[sl, :]).astype(np.float16),
            "bq": np.ascontiguousarray(bq_s[sl]).reshape(DL, 1),
            "bk": np.ascontiguousarray(bk[sl]).reshape(DL, 1),
            "bv": _bv1(bv[sl]),
            "tri": tri,
        })
    return in_maps


def _run(inputs, trace=False):
    from concourse.bass_utils import run_bass_kernel_spmd

    nc = _CACHE.get("nc")
    if nc is None:
        nc = _build_program()
        _CACHE["nc"] = nc
    in_maps = _prepare_in_maps(inputs)
    res = run_bass_kernel_spmd(nc, in_maps, core_ids=list(range(N_CORES)),
                               trace=trace)
    bo = np.asarray(inputs["bo"], np.float32)
    out = np.zeros((B, S, D), np.float32)
    for c in range(N_CORES):
        out[c // 4] += res.results[c]["out"]
    out += bo[None, None, :]
    return out, res


def kernel(**inputs):
    out, _ = _run(inputs, trace=False)
    return out


# revision 11
# speedup vs baseline: 4.9697x; 3.3301x over previous
"""BasinCoupledAttention Trainium2 kernel.

Full inputs -> full output. Sharding: 2-way data parallel over batch x
4-way tensor parallel over heads (4 heads / core, 8 cores total).

Per-core device program (b = core//4, g = core%4, heads 4g..4g+3):
  - x_b^T [1024, 2048] resident in SBUF (d on partitions), fp16
  - q^T, k^T = (Wq_g^T x^T, Wk_g^T x^T) in [d', s] layout, fp16; the
    per-head sigmoid basin gate and 1/sqrt(dh) are folded into Wq_g /
    bq_g on the host
  - v = x Wv_g in [s, d'] layout with a ones-column per head, fp32r
  - attention head-pair interleaved: S^T[j,i] = k_h^T(jb)^T q_h^T (ib)
    via fp16 matmul; exp on ACT (no max subtraction: |scores| <= ~6),
    fp32r out; causal structurally (skip blocks above the diagonal,
    fp16 triangular mask multiply on diagonal squares);
    [out^T; rowsum] accumulated via v1 = [v | 1] fp32r matmuls
  - normalize by broadcast(1/rowsum); out_partial = O^T' Wo per s-chunk
Host sums the 4 per-core partials for each batch and adds bo.
"""

import sys

if "/opt/trn_rl_repo" not in sys.path:
    sys.path.insert(0, "/opt/trn_rl_repo")

import numpy as np

D = 1024          # d_model
S = 2048          # sequence
B = 2             # batch
HL = 4            # heads per core
DL = 256          # d' columns per core (HL * 64)
DH = 64           # head dim
N_CORES = 8

_CACHE = {}

PHASES = ("dma", "v", "qk", "attn", "outp")


def _build_program(reps=1, stop_after="all", loop_only=None,
                   tiny_out=False):
    import concourse.bacc as bacc
    import concourse.mybir as mybir
    import concourse.tile as tile

    f32 = mybir.dt.float32
    f16 = mybir.dt.float16
    f32r = mybir.dt.float32r
    Exp = mybir.ActivationFunctionType.Exp

    nc = bacc.Bacc("TRN2", target_bir_lowering=False, debug=False)

    xt_d = nc.dram_tensor("xt", [D, S], f16, kind="ExternalInput").ap()
    wq_d = nc.dram_tensor("wq", [D, DL], f16, kind="ExternalInput").ap()
    wk_d = nc.dram_tensor("wk", [D, DL], f16, kind="ExternalInput").ap()
    wv_d = nc.dram_tensor("wv", [D, DL], f16, kind="ExternalInput").ap()
    wo_d = nc.dram_tensor("wo", [DL, D], f16, kind="ExternalInput").ap()
    bq_d = nc.dram_tensor("bq", [DL, 1], f32, kind="ExternalInput").ap()
    bk_d = nc.dram_tensor("bk", [DL, 1], f32, kind="ExternalInput").ap()
    # bv1 holds [h, 65]: cols 0..63 = bv bias for that head, col 64 = 1.0
    bv_d = nc.dram_tensor("bv", [1, HL * 65], f32, kind="ExternalInput").ap()
    tri_d = nc.dram_tensor("tri", [128, 128], f16, kind="ExternalInput").ap()
    if tiny_out:
        out_d = nc.dram_tensor("out_big", [S, D], f32).ap()
        out_small = nc.dram_tensor("out", [128, D], f32,
                                   kind="ExternalOutput").ap()
    else:
        out_d = nc.dram_tensor("out", [S, D], f32, kind="ExternalOutput").ap()
        out_small = None

    with tile.TileContext(nc) as tc:
        with (
            tc.tile_pool(name="persist", bufs=1) as pp,
            tc.tile_pool(name="work", bufs=4) as wp,
            tc.tile_pool(name="ps_a", bufs=3, space="PSUM") as ps_a,
            tc.tile_pool(name="ps_av", bufs=2, space="PSUM") as ps_av,
        ):
            st = {}

            def ph_dma():
                xt_sb = [pp.tile([128, S], f16, name=f"xt{k}", tag=f"xt{k}")
                         for k in range(8)]
                for k in range(8):
                    nc.sync.dma_start(out=xt_sb[k],
                                      in_=xt_d[k * 128:(k + 1) * 128, :])
                wq_sb = [pp.tile([128, DL], f16, name=f"wq{k}", tag=f"wq{k}")
                         for k in range(8)]
                wk_sb = [pp.tile([128, DL], f16, name=f"wk{k}", tag=f"wk{k}")
                         for k in range(8)]
                wv_sb = [pp.tile([128, DL], f16, name=f"wv{k}", tag=f"wv{k}")
                         for k in range(8)]
                for k in range(8):
                    sl = slice(k * 128, (k + 1) * 128)
                    nc.sync.dma_start(out=wq_sb[k], in_=wq_d[sl, :])
                    nc.sync.dma_start(out=wk_sb[k], in_=wk_d[sl, :])
                    nc.sync.dma_start(out=wv_sb[k], in_=wv_d[sl, :])
                wo_sb = [pp.tile([128, D], f16, name=f"wo{t}", tag=f"wo{t}")
                        for t in range(2)]
                bq_sb = [pp.tile([128, 1], f32, name=f"bq{t}", tag=f"bq{t}")
                         for t in range(2)]
                bk_sb = [pp.tile([128, 1], f32, name=f"bk{t}", tag=f"bk{t}")
                         for t in range(2)]
                for t in range(2):
                    sl = slice(t * 128, (t + 1) * 128)
                    nc.sync.dma_start(out=wo_sb[t], in_=wo_d[sl, :])
                    nc.sync.dma_start(out=bq_sb[t], in_=bq_d[sl, :])
                    nc.sync.dma_start(out=bk_sb[t], in_=bk_d[sl, :])
                bv_sb = pp.tile([128, HL * 65], f32, name="bv", tag="bv")
                nc.sync.dma_start(out=bv_sb,
                                  in_=bv_d.to_broadcast([128, HL * 65]))
                tri_sb = pp.tile([128, 128], f16, name="tri", tag="tri")
                nc.sync.dma_start(out=tri_sb, in_=tri_d)
                st.update(xt=xt_sb, wq=wq_sb, wk=wk_sb, wv=wv_sb, wo=wo_sb,
                          bq=bq_sb, bk=bk_sb, tri=tri_sb,
                          bv_r=bv_sb.rearrange("p (h c) -> p h c", h=HL))

            def ph_v():
                v1_sb = [pp.tile([128, HL, 65], f32r, name=f"v1_{s_}",
                                 tag=f"v1_{s_}") for s_ in range(16)]
                for sc in range(16):
                    ps_v = ps_a.tile([128, DL], f32, name="ps_v", tag="ps")
                    for kd in range(8):
                        nc.tensor.matmul(
                            ps_v,
                            lhsT=st["xt"][kd][:, sc * 128:(sc + 1) * 128],
                            rhs=st["wv"][kd],
                            start=(kd == 0), stop=(kd == 7),
                        )
                    nc.vector.tensor_tensor(
                        out=v1_sb[sc][:, :, 0:64],
                        in0=ps_v.rearrange("p (h d) -> p h d", h=HL),
                        in1=st["bv_r"][:, :, 0:64],
                        op=mybir.AluOpType.add,
                    )
                    nc.vector.tensor_copy(v1_sb[sc][:, :, 64:65],
                                          st["bv_r"][:, :, 64:65])
                st["v1"] = v1_sb

            def ph_qk():
                qT_sb = [pp.tile([128, S], f16, name=f"qT{t}", tag=f"qT{t}")
                         for t in range(2)]
                kT_sb = [pp.tile([128, S], f16, name=f"kT{t}", tag=f"kT{t}")
                         for t in range(2)]
                for t in range(2):
                    for w_sb, b_sb, dst in ((st["wq"], st["bq"], qT_sb),
                                            (st["wk"], st["bk"], kT_sb)):
                        for nb in range(4):
                            ps_p = ps_a.tile([128, 512], f32, name="ps_p",
                                             tag="ps")
                            for kd in range(8):
                                nc.tensor.matmul(
                                    ps_p,
                                    lhsT=w_sb[kd][:, t * 128:(t + 1) * 128],
                                    rhs=st["xt"][kd][:, nb * 512:(nb + 1) * 512],
                                    start=(kd == 0), stop=(kd == 7),
                                )
                            nc.vector.tensor_scalar_add(
                                out=dst[t][:, nb * 512:(nb + 1) * 512],
                                in0=ps_p, scalar1=b_sb[t],
                            )
                st["qT"], st["kT"] = qT_sb, kT_sb

            def ph_attn():
                oT_sb = st.get("oT")
                if oT_sb is None:
                    oT_sb = [pp.tile([128, S], f16, name=f"oT{t}",
                                     tag=f"oT{t}") for t in range(2)]
                    st["oT"] = oT_sb
                v1_sb, qT_sb, kT_sb = st["v1"], st["qT"], st["kT"]
                tri_sb = st["tri"]
                for t in range(2):           # head pair (2t, 2t+1)
                    for ib in range(4):
                        njb = 4 * ib + 4
                        ps_o = {
                            0: ps_av.tile([65, 512], f32, name="ps_o0",
                                          tag="av"),
                            1: ps_av.tile([65, 512], f32, name="ps_o1",
                                          tag="av"),
                        }
                        pend = []

                        def flush_av(limit):
                            while len(pend) > limit:
                                h, jb, off, a_t = pend.pop(0)
                                nc.tensor.matmul(
                                    ps_o[h][:, off:512],
                                    lhsT=v1_sb[jb][:, 2 * t + h, :],
                                    rhs=a_t[:, 512 * h + off:512 * (h + 1)],
                                    start=(jb == 0), stop=(jb == njb - 1),
                                    skip_group_check=True,
                                )

                        for jb in range(njb):
                            diag = jb >= 4 * ib
                            off = jb * 128 - ib * 512 if diag else 0
                            ps_s = ps_a.tile([128, 1024], f32,
                                             name="ps_s", tag="ps")
                            for h in (0, 1):
                                bp = h * 64
                                nc.tensor.matmul(
                                    ps_s[:, 512 * h:512 * (h + 1)],
                                    lhsT=kT_sb[t][bp:bp + 64,
                                                  jb * 128:(jb + 1) * 128],
                                    rhs=qT_sb[t][bp:bp + 64,
                                                 ib * 512:(ib + 1) * 512],
                                    start=True, stop=True,
                                )
                            a_t = wp.tile([128, 1024], f32r, name="a_t",
                                          tag="a", bufs=4)
                            nc.scalar.activation(a_t[:, off:1024],
                                                 ps_s[:, off:1024], Exp)
                            if diag:
                                for h in (0, 1):
                                    nc.vector.tensor_mul(
                                        a_t[:, 512 * h + off:512 * h + off + 128],
                                        a_t[:, 512 * h + off:512 * h + off + 128],
                                        tri_sb)
                            pend.append((0, jb, off, a_t))
                            pend.append((1, jb, off, a_t))
                            flush_av(3)
                        flush_av(0)
                        for h in (0, 1):
                            bp = h * 64
                            recip = wp.tile([1, 512], f32, name="recip",
                                            tag="rc", bufs=2)
                            nc.vector.reciprocal(recip, ps_o[h][64:65, :])
                            rb = wp.tile([64, 512], f32, name="rb", tag="rb",
                                         bufs=2)
                            nc.gpsimd.partition_broadcast(rb, recip)
                            nc.vector.tensor_mul(
                                oT_sb[t][bp:bp + 64,
                                         ib * 512:(ib + 1) * 512],
                                ps_o[h][0:64, :], rb)

            def ph_outp():
                oT_sb, wo_sb = st["oT"], st["wo"]
                for sc in range(16):
                    o_sb = wp.tile([128, D], f32, name="o_sb", tag="osb",
                                   bufs=3)
                    for n in range(2):
                        ps_f = ps_a.tile([128, 512], f32, name="ps_f",
                                         tag="ps")
                        for t in range(2):
                            nc.tensor.matmul(
                                ps_f,
                                lhsT=oT_sb[t][:, sc * 128:(sc + 1) * 128],
                                rhs=wo_sb[t][:, n * 512:(n + 1) * 512],
                                start=(t == 0), stop=(t == 1),
                            )
                        nc.vector.tensor_copy(o_sb[:, n * 512:(n + 1) * 512],
                                              ps_f)
                    nc.sync.dma_start(out=out_d[sc * 128:(sc + 1) * 128, :],
                                      in_=o_sb)
                if out_small is not None:
                    nc.sync.dma_start(out=out_small, in_=o_sb)

            phase_fns = {"dma": ph_dma, "v": ph_v, "qk": ph_qk,
                         "attn": ph_attn, "outp": ph_outp}
            if stop_after == "all":
                active = list(PHASES)
            else:
                active = list(PHASES[:PHASES.index(stop_after) + 1])

            def make_loop():
                return tc.For_i(0, reps, 1,
                                hint_engines=(mybir.EngineType.PE,
                                              mybir.EngineType.Activation,
                                              mybir.EngineType.DVE))

            if loop_only is None:
                if reps == 1:
                    for name in active:
                        phase_fns[name]()
                else:
                    with make_loop():
                        for name in active:
                            phase_fns[name]()
            else:
                assert loop_only in active
                for name in active[:active.index(loop_only)]:
                    phase_fns[name]()
                with make_loop():
                    phase_fns[loop_only]()
                for name in active[active.index(loop_only) + 1:]:
                    phase_fns[name]()

    nc.compile()
    return nc


def _bv1(bv_slice):
    """[256] head-local v bias -> [1, 4*65] with a 1.0 ones-column."""
    out = np.zeros((1, HL * 65), np.float32)
    for h in range(HL):
        out[0, h * 65:h * 65 + 64] = bv_slice[h * 64:(h + 1) * 64]
        out[0, h * 65 + 64] = 1.0
    return out


def _prepare_in_maps(inputs):
    x = np.asarray(inputs["x"], np.float32)
    basin = np.asarray(inputs["basin"], np.float32)
    Wq = np.asarray(inputs["Wq"], np.float32)
    bq = np.asarray(inputs["bq"], np.float32)
    Wk = np.asarray(inputs["Wk"], np.float32)
    bk = np.asarray(inputs["bk"], np.float32)
    Wv = np.asarray(inputs["Wv"], np.float32)
    bv = np.asarray(inputs["bv"], np.float32)
    Wo = np.asarray(inputs["Wo"], np.float32)
    Wb = np.asarray(inputs["Wb"], np.float32)
    bb = np.asarray(inputs["bb"], np.float32)

    gate = 1.0 / (1.0 + np.exp(-(basin @ Wb + bb)))          # [16]
    scale = (gate / np.sqrt(float(DH))).astype(np.float32)   # [16]
    colscale = np.repeat(scale, DH)                          # [1024]
    Wq_s = (Wq * colscale[None, :]).astype(np.float32)
    bq_s = (bq * colscale).astype(np.float32)

    xt_all = [np.ascontiguousarray(x[b].T) for b in range(B)]
    tri = np.ascontiguousarray(np.triu(np.ones((128, 128), np.float16)))

    in_maps = []
    for c in range(N_CORES):
        b, g = divmod(c, 4)
        sl = slice(g * DL, (g + 1) * DL)
        in_maps.append({
            "xt": xt_all[b].astype(np.float16),
            "wq": np.ascontiguousarray(Wq_s[:, sl]).astype(np.float16),
            "wk": np.ascontiguousarray(Wk[:, sl]).astype(np.float16),
            "wv": np.ascontiguousarray(Wv[:, sl]).astype(np.float16),
            "wo": np.ascontiguousarray(Wo[sl, :]).astype(np.float16),
            "bq": np.ascontiguousarray(bq_s[sl]).reshape(DL, 1),
            "bk": np.ascontiguousarray(bk[sl]).reshape(DL, 1),
            "bv": _bv1(bv[sl]),
            "tri": tri,
        })
    return in_maps


def _run(inputs, trace=False):
    from concourse.bass_utils import run_bass_kernel_spmd

    nc = _CACHE.get("nc")
    if nc is None:
        nc = _build_program()
        _CACHE["nc"] = nc
    in_maps = _prepare_in_maps(inputs)
    res = run_bass_kernel_spmd(nc, in_maps, core_ids=list(range(N_CORES)),
                               trace=trace)
    bo = np.asarray(inputs["bo"], np.float32)
    out = np.zeros((B, S, D), np.float32)
    for c in range(N_CORES):
        out[c // 4] += res.results[c]["out"]
    out += bo[None, None, :]
    return out, res


def kernel(**inputs):
    out, _ = _run(inputs, trace=False)
    return out
